# revision 9
# baseline (speedup 1.0000x reference)
# Trainium2 Bass kernel for nn_EqPropNetwork (equilibrium-propagation relaxation).
#
# Math (per reference.py):
#   c_h = x @ W1 + b1                                  [B, HID]  (constant over steps)
#   repeat T times:
#     psi = y @ W2.T ; phi = h @ W2
#     h'  = clip(0.5*h + 0.5*c_h + 0.5*psi, 0, 1)
#     y'  = clip(0.25*y + 0.5*phi + 0.5*b2 + 0.25*onehot(target), 0, 1)
#   out = concat(h, y)                                  [B, HID+OUT]
#
# Mapping (per core, B_loc = 4096, pure data parallel over 8 cores):
#   * Loop state s := h + c_h, feature-major fp16 (partition = HID chunk,
#     free = batch).  Per step, per (chunk c, batch block rb) u-tile
#     [128,1024] in PSUM: u = 0.5*s (identity matmul) + 0.5*y@W2T
#     (row-group-packed K=10 matmuls).  Readback h' = clip01(u), then
#     s' = h' + c_h, split across ACT / DVE / GpSimd via a path table.
#   * y is blocked: partition 32j+i holds y[i, 1024j:1024(j+1)].  One
#     [128,1024] PSUM tile per step accumulates 0.5*s@W2 (col-packed) +
#     0.25*y + dbar, with dbar = 0.25*onehot + 0.5*b2 - 0.5*(c_h@W2).
#   * Inputs x/h/W1 are fp16-converted on the host (halves DMA, no device
#     casts); h output is written fp16 and widened on the host.
import os
import sys

import numpy as np

if "/opt/trn_rl_repo" not in sys.path:
    sys.path.insert(0, "/opt/trn_rl_repo")

N_CORES = 8
B, IN, HID, OUT = 32768, 784, 512, 10
BLOC = B // N_CORES  # 4096
NBLK = BLOC // 1024  # 4 batch blocks of 1024
KIN = 7              # IN chunks of 112
KC = IN // KIN       # 112
HCH = HID // 128     # 4 hidden chunks
NT = HCH * NBLK      # 16 u-tiles per step

# packed fp16 const tile column offsets
C_HALFI = 0          # [128, 128] 0.5*I128
C_I10Q = 128         # [128, 10] 0.25*I10 at 4 row offsets
C_I10D = 138         # [128, 10] I10 at 4 row offsets
C_R1 = 148           # [128, 128] rep: R1[i, 32j+i]=1 (i<10)
C_R2 = 276           # [128, 128] rep: R2[32j, 32j+i]=1 (i<10)
C_ONES = 404         # [1, 512] ones row (rank-1 bias matmul)
CF16_W = 916

# readback path per u-tile ti = 4*c + rb:
#   "AD": ACT relu -> DVE stt (min1, +c_h)
#   "AA": ACT relu(1-u) -> ACT relu(1-.) = clip01(u) -> DVE tt +c_h (2x mode)
#   "D":  DVE ts clamp01 -> DVE tt +c_h
#   "DG": DVE ts clamp01 -> GPS tt +c_h
PATHS = ["AD", "AA", "DG", "AD",
         "DG", "D", "AD", "AA",
         "DG", "AD", "DG", "D",
         "AD", "AA", "DG", "AA"]

_BUILT = {}


def _build(T):
    import concourse.bass as bass
    from concourse import bacc, mybir
    from concourse.tile import TileContext

    f32 = mybir.dt.float32
    f16 = mybir.dt.float16
    i32 = mybir.dt.int32
    Alu = mybir.AluOpType
    Act = mybir.ActivationFunctionType

    nc = bacc.Bacc("TRN2", target_bir_lowering=False)

    xT = nc.declare_dram_parameter("xT", [IN, BLOC], f16, isOutput=False)
    hT = nc.declare_dram_parameter("hT", [HID, BLOC], f16, isOutput=False)
    yT = nc.declare_dram_parameter("yT", [OUT, BLOC], f32, isOutput=False)
    W1 = nc.declare_dram_parameter("W1", [IN, HID], f16, isOutput=False)
    W2 = nc.declare_dram_parameter("W2", [HID, OUT], f32, isOutput=False)
    b1 = nc.declare_dram_parameter("b1", [HID, 1], f32, isOutput=False)
    b2 = nc.declare_dram_parameter("b2", [OUT, 1], f32, isOutput=False)
    tgt = nc.declare_dram_parameter("tgt", [BLOC, 2], i32, isOutput=False)
    cst16 = nc.declare_dram_parameter("cst16", [128, CF16_W], f16, isOutput=False)
    cst32 = nc.declare_dram_parameter("cst32", [128, 8], f32, isOutput=False)

    hT_out = nc.declare_dram_parameter("hT_out", [HID, BLOC], f16, isOutput=True)
    yT_out = nc.declare_dram_parameter("yT_out", [OUT, BLOC], f32, isOutput=True)

    with TileContext(nc) as tc:
        with (
            tc.tile_pool(name="const", bufs=1) as constp,
            tc.tile_pool(name="ch", bufs=1) as chp,
            tc.tile_pool(name="state", bufs=2) as sp,
            tc.tile_pool(name="ypool", bufs=2) as yp,
        ):
            cf16 = constp.tile([128, CF16_W], f16, tag="cf16", name="cf16")
            cf32 = constp.tile([128, 8], f32, tag="cf32", name="cf32")
            dbar = constp.tile([128, 1024], f16, tag="dbar", name="dbar")
            cb1 = constp.tile([128, HCH], f32, tag="cb1", name="cb1")
            w2t = constp.tile([128, HCH * 128], f16, tag="w2t", name="w2t")
            w2cs = constp.tile([128, HCH * OUT], f16, tag="w2cs", name="w2cs")
            nc.sync.dma_start(out=cf16[:], in_=cst16[:])
            nc.sync.dma_start(out=cf32[:], in_=cst32[:])
            nc.sync.dma_start(
                out=cb1.rearrange("p (c o) -> p c o", c=HCH),
                in_=b1.rearrange("(c p) o -> p c o", c=HCH),
            )

            halfI_t = cf16[:, C_HALFI:C_HALFI + 128]
            I10q_t = cf16[:, C_I10Q:C_I10Q + OUT]
            I10d_t = cf16[:, C_I10D:C_I10D + OUT]
            R1_t = cf16[:, C_R1:C_R1 + 128]
            R2_t = cf16[:, C_R2:C_R2 + 128]
            ones_t = cf16[:, C_ONES:C_ONES + 512]
            idxf_t = cf32[:, 0:1]
            b1c = [cb1[:, c:c + 1] for c in range(HCH)]
            W2Tr = [w2t[:, 128 * c:128 * (c + 1)] for c in range(HCH)]
            W2c = [w2cs[:, OUT * c:OUT * (c + 1)] for c in range(HCH)]

            ch = chp.tile([128, HCH * BLOC], f16, tag="ch", name="ch")
            chv = [ch[:, BLOC * c:BLOC * (c + 1)] for c in range(HCH)]

            with (
                tc.tile_pool(name="x16p", bufs=1) as x16p,
                tc.tile_pool(name="mst", bufs=3) as mstp,
                tc.tile_pool(name="spsum", bufs=6, space="PSUM") as spsum,
            ):
                # ---------- inputs ----------
                x16 = x16p.tile([128, KIN * BLOC], f16, tag="x16", name="x16")
                w1_16 = x16p.tile([128, KIN * HID], f16, tag="w1_16", name="w1_16")
                s0 = sp.tile([128, HCH * BLOC], f16, tag="s", name="s")
                for k in range(KIN):
                    nc.sync.dma_start(
                        out=w1_16[:KC, HID * k:HID * (k + 1)],
                        in_=W1[KC * k:KC * (k + 1), :],
                    )
                for k in range(KIN):
                    nc.sync.dma_start(
                        out=x16[:KC, BLOC * k:BLOC * (k + 1)],
                        in_=xT[KC * k:KC * (k + 1), :],
                    )
                # h0 lands directly in the s0 tile; c_h added in place below
                nc.sync.dma_start(
                    out=s0.rearrange("p (c o) -> p c o", c=HCH),
                    in_=hT.rearrange("(c p) o -> p c o", c=HCH),
                )

                # ---------- W2-derived forms (overlap with x DMA) ----------
                # W2c = 0.5*W2 chunks [128, 4*10]
                st = mstp.tile([128, HCH * OUT], f32, tag="mst2", name="mst2")
                nc.vector.memset(st[:], 0.0)
                nc.sync.dma_start(
                    out=st.rearrange("p (c i) -> p c i", c=HCH),
                    in_=W2.rearrange("(c p) i -> p c i", c=HCH),
                )
                nc.vector.tensor_scalar_mul(w2cs[:], st[:], 0.5)
                # W2Tr_c[32r+i, m] = 0.5*W2[128c+m, i]: rows 0-9 then R1 rep.
                for c in range(HCH):
                    st = mstp.tile([128, 128], f32, tag="mst2", name="mst2")
                    nc.vector.memset(st[:], 0.0)
                    nc.sync.dma_start(
                        out=st[:OUT, :],
                        in_=W2[128 * c:128 * (c + 1), :].rearrange("m i -> i m"),
                    )
                    st16 = mstp.tile([128, 128], f16, tag="mst2b", name="mst2b")
                    nc.vector.tensor_scalar_mul(st16[:OUT, :], st[:OUT, :], 0.5)
                    ps = spsum.tile([128, 128], f32, tag="spr", name="spr", bufs=2)
                    nc.tensor.matmul(
                        ps[:], R1_t[:OUT, :], st16[:OUT, :],
                        start=True, stop=True, tile_position=(0, 0),
                    )
                    nc.vector.tensor_copy(W2Tr[c], ps[:])
                # -0.5*b2 as a [1, OUT] f16 row (rank-1 matmul operand)
                stb = mstp.tile([128, 16], f32, tag="b2st", name="b2st", bufs=1)
                b2l = mstp.tile([128, 16], f16, tag="b2l", name="b2l", bufs=1)
                nc.vector.memset(stb[:], 0.0)
                nc.sync.dma_start(
                    out=stb[0:1, 0:OUT], in_=b2.rearrange("i o -> o i")
                )
                nc.vector.tensor_scalar_mul(b2l[0:1, 0:OUT], stb[0:1, 0:OUT], -0.5)

                # target one-hot staging (R2 bcast later)
                t32 = mstp.tile([128, 1024], i32, tag="tgtst", name="tgtst", bufs=1)
                nc.vector.memset(t32[:], 0)
                for j in range(NBLK):
                    nc.sync.dma_start(
                        out=t32[32 * j:32 * j + 1, :],
                        in_=tgt[1024 * j:1024 * (j + 1), 0:1].rearrange("a b -> b a"),
                    )
                tf16 = mstp.tile([128, 1024], f16, tag="tgtf16", name="tgtf16", bufs=1)
                nc.vector.tensor_copy(tf16[:], t32[:])

                # y0 blocked
                yst = mstp.tile([128, 1024], f32, tag="y0st", name="y0st", bufs=1)
                nc.vector.memset(yst[:], 0.0)
                for j in range(NBLK):
                    nc.sync.dma_start(
                        out=yst[32 * j:32 * j + OUT, :],
                        in_=yT[:, 1024 * j:1024 * (j + 1)],
                    )
                ycur = yp.tile([128, 1024], f16, tag="yblk", name="yblk")
                nc.vector.tensor_copy(ycur[:], yst[:])

                # ---------- c_h = x@W1 + b1 (K=112 chunks, k-inner) ----------
                for c in range(HCH):
                    for blk in range(BLOC // 512):
                        psc = spsum.tile([128, 512], f32, tag="spsum", name="spsum")
                        for k in range(KIN):
                            nc.tensor.matmul(
                                psc[:],
                                w1_16[:KC, HID * k + 128 * c:HID * k + 128 * (c + 1)],
                                x16[:KC, BLOC * k + 512 * blk:BLOC * k + 512 * (blk + 1)],
                                start=(k == 0),
                                stop=(k == KIN - 1),
                                tile_position=(0, 0),
                            )
                        nc.scalar.activation(
                            chv[c][:, 512 * blk:512 * (blk + 1)],
                            psc[:],
                            Act.Identity,
                            bias=b1c[c],
                            scale=1.0,
                        )

                # ---------- dbar = 0.25*onehot + 0.5*b2 - 0.5*(c_h@W2) ----------
                ublk = mstp.tile([128, 1024], f16, tag="ublk", name="ublk", bufs=1)
                for half in range(2 * NBLK):
                    j, hf = half // 2, half % 2
                    ps = spsum.tile([128, 512], f32, tag="spsum", name="spsum")
                    for c in range(HCH):
                        nc.tensor.matmul(
                            ps[32 * j:32 * j + OUT, :],
                            W2c[c],
                            chv[c][:, 1024 * j + 512 * hf:1024 * j + 512 * (hf + 1)],
                            start=(c == 0),
                            stop=False,
                            tile_position=(0, 32 * j),
                        )
                    nc.tensor.matmul(
                        ps[32 * j:32 * j + OUT, :],
                        b2l[0:1, 0:OUT],
                        ones_t[0:1, 0:512],
                        start=False,
                        stop=True,
                        tile_position=(0, 32 * j),
                    )
                    nc.scalar.activation(
                        ublk[32 * j:32 * j + OUT, 512 * hf:512 * (hf + 1)],
                        ps[32 * j:32 * j + OUT, :],
                        Act.Identity,
                        bias=0.0,
                        scale=-1.0,
                    )

                eq = mstp.tile([128, 1024], f16, tag="eqt", name="eqt", bufs=1)
                for hf in range(2):
                    ps = spsum.tile([128, 512], f32, tag="spsum", name="spsum")
                    nc.tensor.matmul(
                        ps[:], R2_t, tf16[:, 512 * hf:512 * (hf + 1)],
                        start=True, stop=True, tile_position=(0, 0),
                    )
                    nc.vector.tensor_scalar(
                        eq[:, 512 * hf:512 * (hf + 1)], ps[:],
                        idxf_t, 0.25, Alu.is_equal, Alu.mult,
                    )
                nc.vector.tensor_tensor(dbar[:], eq[:], ublk[:], Alu.add)

                # ---------- s0 = h0 + c_h (in place; h0 already in s0) ----------
                for c in range(HCH):
                    nc.vector.tensor_tensor(
                        s0[:, BLOC * c:BLOC * (c + 1)],
                        s0[:, BLOC * c:BLOC * (c + 1)],
                        chv[c][:],
                        Alu.add,
                    )
                scur = s0

            # ---------- relaxation loop ----------
            with (
                tc.tile_pool(name="pu", bufs=3, space="PSUM") as pup,
                tc.tile_pool(name="py", bufs=1, space="PSUM") as pyp,
                tc.tile_pool(name="tmp", bufs=6) as tmpp,
                tc.tile_pool(name="hout", bufs=1) as houtp,
            ):
                hout = None
                for t in range(T):
                    last = t == T - 1
                    sv = [scur[:, BLOC * c:BLOC * (c + 1)] for c in range(HCH)]
                    # ---- y update (one [128,1024] PSUM tile, col-packed) ----
                    psy = pyp.tile([128, 1024], f32, tag="py", name="py")
                    for hf in range(2):
                        sl = slice(512 * hf, 512 * (hf + 1))
                        for c in range(HCH):
                            for j in range(NBLK):
                                nc.tensor.matmul(
                                    psy[32 * j:32 * j + OUT, sl],
                                    W2c[c],
                                    sv[c][:, 1024 * j + 512 * hf:
                                          1024 * j + 512 * (hf + 1)],
                                    start=(c == 0),
                                    stop=False,
                                    tile_position=(0, 32 * j),
                                )
                        for j in range(NBLK):
                            nc.tensor.matmul(
                                psy[32 * j:32 * j + OUT, sl],
                                I10q_t[32 * j:32 * j + OUT, :],
                                ycur[32 * j:32 * j + OUT, sl],
                                start=False,
                                stop=False,
                                tile_position=(32 * j, 32 * j),
                            )
                        for j in range(NBLK):
                            nc.tensor.matmul(
                                psy[32 * j:32 * j + OUT, sl],
                                I10d_t[32 * j:32 * j + OUT, :],
                                dbar[32 * j:32 * j + OUT, sl],
                                start=False,
                                stop=True,
                                tile_position=(32 * j, 32 * j),
                            )
                    ynext = yp.tile([128, 1024], f16, tag="yblk", name="yblk")
                    nc.vector.tensor_scalar(
                        ynext[:], psy[:], 0.0, 1.0, Alu.max, Alu.min
                    )

                    # ---- h update: rolling groups of 3 u-tiles ----
                    if last:
                        hout = houtp.tile(
                            [128, HCH * BLOC], f16, tag="hout", name="hout"
                        )
                        snext = None
                    else:
                        snext = sp.tile([128, HCH * BLOC], f16, tag="s", name="s")

                    tiles = list(range(NT))
                    for g0 in range(0, NT, 3):
                        group = tiles[g0:g0 + 3]
                        pus = {}
                        # identity MMs back-to-back (full array, same weights)
                        for ti in group:
                            c, rb = ti // 4, ti % 4
                            pu = pup.tile([128, 1024], f32, tag="pu", name="pu")
                            pus[ti] = pu
                            for hf in range(2):
                                nc.tensor.matmul(
                                    pu[:, 512 * hf:512 * (hf + 1)],
                                    halfI_t,
                                    sv[c][:, 1024 * rb + 512 * hf:
                                          1024 * rb + 512 * (hf + 1)],
                                    start=True, stop=False, tile_position=(0, 0),
                                )
                        # psi MMs row-group packed (distinct rb per wave)
                        for hf in range(2):
                            for ti in group:
                                c, rb = ti // 4, ti % 4
                                nc.tensor.matmul(
                                    pus[ti][:, 512 * hf:512 * (hf + 1)],
                                    W2Tr[c][32 * rb:32 * rb + OUT, :],
                                    ycur[32 * rb:32 * rb + OUT,
                                         512 * hf:512 * (hf + 1)],
                                    start=False, stop=True,
                                    tile_position=(32 * rb, 0),
                                )
                        # readback
                        for ti in group:
                            c, rb = ti // 4, ti % 4
                            pu = pus[ti]
                            cols = slice(1024 * rb, 1024 * (rb + 1))
                            ccols = slice(BLOC * c + 1024 * rb,
                                          BLOC * c + 1024 * (rb + 1))
                            path = PATHS[ti]
                            if last:
                                # h_T = clip01(u), fp16, no +c_h
                                if path == "AD":
                                    r = tmpp.tile(
                                        [128, 1024], f16, tag="tmp", name="tmp"
                                    )
                                    nc.scalar.activation(r[:], pu[:], Act.Relu)
                                    nc.vector.tensor_scalar_min(
                                        hout[:, ccols], r[:], 1.0
                                    )
                                elif path == "AA":
                                    r = tmpp.tile(
                                        [128, 1024], f16, tag="tmp", name="tmp"
                                    )
                                    nc.scalar.activation(
                                        r[:], pu[:], Act.Relu, bias=1.0, scale=-1.0
                                    )
                                    nc.scalar.activation(
                                        hout[:, ccols], r[:], Act.Relu,
                                        bias=1.0, scale=-1.0,
                                    )
                                else:
                                    nc.vector.tensor_scalar(
                                        hout[:, ccols], pu[:], 0.0, 1.0,
                                        Alu.max, Alu.min,
                                    )
                            elif path == "AD":
                                r = tmpp.tile([128, 1024], f16, tag="tmp", name="tmp")
                                nc.scalar.activation(r[:], pu[:], Act.Relu)
                                nc.vector.scalar_tensor_tensor(
                                    snext[:, ccols], r[:], 1.0,
                                    chv[c][:, cols], Alu.min, Alu.add,
                                )
                            elif path == "AA":
                                # clip01(u) = relu(1 - relu(1 - u)), both on ACT
                                r = tmpp.tile([128, 1024], f16, tag="tmp", name="tmp")
                                nc.scalar.activation(
                                    r[:], pu[:], Act.Relu, bias=1.0, scale=-1.0
                                )
                                rm = tmpp.tile([128, 1024], f16, tag="tmp2", name="tmp2")
                                nc.scalar.activation(
                                    rm[:], r[:], Act.Relu, bias=1.0, scale=-1.0
                                )
                                nc.vector.tensor_tensor(
                                    snext[:, ccols], rm[:], chv[c][:, cols], Alu.add
                                )
                            else:  # "D" / "DG"
                                r = tmpp.tile([128, 1024], f16, tag="tmp", name="tmp")
                                nc.vector.tensor_scalar(
                                    r[:], pu[:], 0.0, 1.0, Alu.max, Alu.min
                                )
                                eng = nc.vector if path == "D" else nc.gpsimd
                                eng.tensor_tensor(
                                    snext[:, ccols], r[:], chv[c][:, cols], Alu.add
                                )
                    if not last:
                        scur = snext
                    ycur = ynext

                # ---------- tail ----------
                for c in range(HCH):
                    nc.sync.dma_start(
                        out=hT_out[128 * c:128 * (c + 1), :],
                        in_=hout[:, BLOC * c:BLOC * (c + 1)],
                    )
                yst2 = tmpp.tile([128, 1024], f32, tag="yo", name="yo", bufs=1)
                nc.vector.tensor_copy(yst2[:], ycur[:])
                for j in range(NBLK):
                    nc.sync.dma_start(
                        out=yT_out[:, 1024 * j:1024 * (j + 1)],
                        in_=yst2[32 * j:32 * j + OUT, :],
                    )

    if not nc.is_finalized():
        nc.finalize()
    return nc


def _consts():
    cst16 = np.zeros((128, CF16_W), dtype=np.float16)
    cst16[:, C_HALFI:C_HALFI + 128] = 0.5 * np.eye(128, dtype=np.float16)
    cst16[0, C_ONES:C_ONES + 512] = 1.0
    cst32 = np.zeros((128, 8), dtype=np.float32)
    cst32[:, 0] = -1.0
    for j in range(NBLK):
        for i in range(OUT):
            cst16[32 * j + i, C_I10Q + i] = 0.25
            cst16[32 * j + i, C_I10D + i] = 1.0
            cst16[i, C_R1 + 32 * j + i] = 1.0
            cst16[32 * j, C_R2 + 32 * j + i] = 1.0
            cst32[32 * j + i, 0] = float(i)
    return cst16, cst32


def kernel(**inputs):
    from concourse import bass_utils

    x = np.asarray(inputs["x"], dtype=np.float32)
    h0 = np.asarray(inputs["h_init"], dtype=np.float32)
    y0 = np.asarray(inputs["y_init"], dtype=np.float32)
    W1 = np.asarray(inputs["W1"], dtype=np.float32)
    W2 = np.ascontiguousarray(np.asarray(inputs["W2"], dtype=np.float32))
    b1 = np.ascontiguousarray(
        np.asarray(inputs["b1"], dtype=np.float32).reshape(HID, 1)
    )
    b2 = np.ascontiguousarray(
        np.asarray(inputs["b2"], dtype=np.float32).reshape(OUT, 1)
    )
    target = np.ascontiguousarray(inputs["target"])
    T = int(inputs["T"])

    # layout prep (sharding): feature-major fp16 slices per core
    xT = np.ascontiguousarray(x.T.astype(np.float16))    # [IN, B]
    hT = np.ascontiguousarray(h0.T.astype(np.float16))   # [HID, B]
    yT = np.ascontiguousarray(y0.T)                      # [OUT, B] f32
    W1_16 = np.ascontiguousarray(W1.astype(np.float16))
    if target.dtype == np.int64:
        tgt32 = target.view(np.int32).reshape(B, 2)  # int64 -> (lo, hi) pairs
    else:
        tgt32 = np.zeros((B, 2), dtype=np.int32)
        tgt32[:, 0] = target

    key = T
    if key not in _BUILT:
        _BUILT[key] = _build(T)
    nc = _BUILT[key]

    cst16, cst32 = _consts()
    in_maps = []
    for k in range(N_CORES):
        sl = slice(k * BLOC, (k + 1) * BLOC)
        in_maps.append({
            "xT": np.ascontiguousarray(xT[:, sl]),
            "hT": np.ascontiguousarray(hT[:, sl]),
            "yT": np.ascontiguousarray(yT[:, sl]),
            "W1": W1_16, "W2": W2, "b1": b1, "b2": b2,
            "tgt": np.ascontiguousarray(tgt32[sl]),
            "cst16": cst16, "cst32": cst32,
        })

    res = bass_utils.run_bass_kernel_spmd(nc, in_maps, list(range(N_CORES)))
    globals()["_LAST_RESULTS"] = res

    out = np.empty((B, HID + OUT), dtype=np.float32)
    for k in range(N_CORES):
        sl = slice(k * BLOC, (k + 1) * BLOC)
        out[sl, :HID] = np.asarray(res.results[k]["hT_out"]).T.astype(np.float32)
        out[sl, HID:] = np.asarray(res.results[k]["yT_out"]).T
    return out
